# revision 1
# baseline (speedup 1.0000x reference)
"""Contrastive Gram-matrix loss kernel for 8 Trainium2 NeuronCores.

Math (see reference):
    x = input.reshape(B, C, W);  G_b = x_b @ x_b.T / (C*W)        [B, C, C]
    D[i,j] = ||G_i - G_j||_F^2 / C^2  (via sq_i + sq_j - 2*dot_ij)
    Sn[i]  = sum_{neg j} exp(alpha - D[i,j])
    loss   = sum_{pos i<j} relu(log(Sn_i + Sn_j) + D_ij)^2 / (2 P)

Sharding: data-parallel over B (8 samples/core).  Each core computes its 8
Grams with PE matmuls (inputs cast fp32->bf16/fp8 during the DMA), the tiny
[B, C*C] Gram matrix is AllGathered in bf16, and every core redundantly
computes the [B,B] distance matrix + masked reduction; host reads core 0's
scalar.  The loss is insensitive to Gram precision here (D ~ 3e-8 while all
other terms are O(1)), so low-precision Grams are safe.
"""

import numpy as np

import concourse.bass as bass
import concourse.bacc as bacc
import concourse.mybir as mybir
import concourse.tile as tile
from concourse.bass_utils import run_bass_kernel_spmd
from concourse.masks import make_identity

F32 = mybir.dt.float32
BF16 = mybir.dt.bfloat16
FP8 = mybir.dt.float8e4

B = 64          # batch
C = 64          # channels
W = 128 * 128   # flattened spatial
N_CORES = 8
SAMP_PER_CORE = B // N_CORES      # 8
PAIRS = SAMP_PER_CORE // 2        # 4 pair-tiles of [128, W]
CHUNK = 128
N_CHUNKS = W // CHUNK             # 128
ALPHA = 1.0
CC = C * C                        # 4096
DOT_CHUNKS = CC // CHUNK          # 32

DT1 = BF16      # phase-1 on-chip dtype (validated on HW; FP8 optional)

_CACHED = {}


def build_program(w=W, variant="full", reps=1, p2reps=1, agreps=1, dt1=None,
                  tail_cut=99):
    # variant: "loads" (DMA-cast only), "loadsf32" (plain f32 DMA),
    #          "grams" (+transpose/matmul/extract), "gramsag" (+AllGather),
    #          "full" (everything)
    # reps/p2reps/agreps: repetition counts (timing amplification only)
    # dt1: phase-1 on-chip dtype (BF16 or FP8)
    if dt1 is None:
        dt1 = DT1
    n_chunks = w // CHUNK
    gram_scale = 1.0 / (C * w)
    nc = bacc.Bacc("TRN2", target_bir_lowering=False, debug=False,
                   num_devices=N_CORES)

    x = nc.dram_tensor("x", [SAMP_PER_CORE, C, w], F32, kind="ExternalInput")
    tgt = nc.dram_tensor("tgt", [1, B], F32, kind="ExternalInput")
    loss_out = nc.dram_tensor("loss", [1, 1], F32, kind="ExternalOutput")

    with tile.TileContext(nc) as tc:
        with tc.tile_pool(name="const", bufs=1) as constp, \
             tc.tile_pool(name="msk", bufs=1) as mskp, \
             tc.tile_pool(name="dram", bufs=1, space="DRAM") as dramp:

            # ---- constants ----
            ident_bf = constp.tile([128, 128], BF16)
            make_identity(nc, ident_bf[:])
            if dt1 is BF16:
                ident_p1 = ident_bf
            else:
                ident_p1 = constp.tile([128, 128], dt1, name="ident_p1")
                make_identity(nc, ident_p1[:])
            ident_f32 = constp.tile([64, 64], F32)
            make_identity(nc, ident_f32[:])
            ones_row = constp.tile([1, B], F32)       # ones [1, 64]
            nc.gpsimd.memset(ones_row[:], 1.0)
            ones_all = constp.tile([B, B], F32)       # ones [64, 64]
            nc.gpsimd.memset(ones_all[:], 1.0)
            alpha_col = constp.tile([B, 1], F32)      # Exp bias; also ones col
            nc.gpsimd.memset(alpha_col[:], ALPHA)
            zero_col = constp.tile([B, 1], F32)
            nc.gpsimd.memset(zero_col[:], 0.0)

            ag_in = dramp.tile([SAMP_PER_CORE, CC], BF16)
            ag_out = dramp.tile([B, CC], BF16, addr_space="Shared")

            # ---- masks from target (independent of x: emitted first so the
            # scheduler hides them under the phase-1 DMAs) ----
            if variant == "full":
                with tc.tile_pool(name="mps", bufs=1, space="PSUM") as mps:
                    tcol = mskp.tile([B, 1], F32)
                    nc.sync.dma_start(tcol[:],
                                      tgt.rearrange("o b -> (o b)")[:, None])
                    trow = mskp.tile([1, B], F32)
                    nc.sync.dma_start(trow[:], tgt[:])
                    tbc_ps = mps.tile([B, B], F32)
                    nc.tensor.matmul(tbc_ps[:], ones_row[:], trow[:],
                                     start=True, stop=True)
                    same = mskp.tile([B, B], F32)
                    nc.vector.tensor_scalar(same[:], tbc_ps[:], tcol[:], None,
                                            op0=mybir.AluOpType.is_equal)
                pos = mskp.tile([B, B], F32)
                nc.gpsimd.affine_select(
                    out=pos[:], in_=same[:],
                    compare_op=mybir.AluOpType.is_gt, fill=0.0,
                    base=0, pattern=[[1, B]], channel_multiplier=-1)
                neg = mskp.tile([B, B], F32)
                nc.vector.tensor_scalar(neg[:], same[:], -1.0, 1.0,
                                        op0=mybir.AluOpType.mult,
                                        op1=mybir.AluOpType.add)
                stats = mskp.tile([B, 2], F32)
                nc.vector.reduce_sum(stats[:, 1:2], pos[:],
                                     axis=mybir.AxisListType.X)

            # ---------------- Phase 1: local Grams -----------------
            with tc.tile_pool(name="xin", bufs=(4 if variant == "loads" else 2)) as xpool, \
                 tc.tile_pool(name="xinf", bufs=2) as xpoolf, \
                 tc.tile_pool(name="xt", bufs=4) as xtpool, \
                 tc.tile_pool(name="gext", bufs=2) as gext, \
                 tc.tile_pool(name="pt", bufs=4, space="PSUM") as ptpool, \
                 tc.tile_pool(name="gps", bufs=2, space="PSUM") as gpool:

                for rp in range(reps * PAIRS):
                    r, p = divmod(rp, PAIRS)
                    if variant == "loadsf32":
                        # diagnostic: plain f32 HWDGE load, no cast
                        xf_pair = xpoolf.tile([128, w], F32, tag="xfpair")
                        nc.sync.dma_start(
                            xf_pair[:],
                            x[2 * p:2 * p + 2].rearrange("s c w -> (s c) w"),
                        )
                        if rp == 0:
                            probe_acc = gext.tile([128, PAIRS], F32,
                                                  name="probe_acc")
                            _CACHED["probe_acc"] = probe_acc
                        pa = _CACHED["probe_acc"]
                        nc.vector.tensor_copy(pa[:, p:p + 1],
                                              xf_pair[:, p:p + 1])
                        if rp == reps * PAIRS - 1:
                            nc.sync.dma_start(
                                loss_out[:],
                                _CACHED.pop("probe_acc")[0:1, 0:1])
                        continue
                    xt_pair = xpool.tile([128, w], dt1, tag="xpair")
                    # one contiguous 8 MB read: two samples stacked on the
                    # partition axis; SWDGE casts fp32 -> dt1 in flight
                    nc.gpsimd.dma_start(
                        xt_pair[:],
                        x[2 * p:2 * p + 2].rearrange("s c w -> (s c) w"),
                    )
                    if variant == "loads":
                        # consume one column so the load isn't dead
                        if rp == 0:
                            probe_acc = gext.tile([128, PAIRS], F32,
                                                  name="probe_acc")
                            _CACHED["probe_acc"] = probe_acc
                        pa = _CACHED["probe_acc"]
                        nc.vector.tensor_copy(pa[:, p:p + 1],
                                              xt_pair[:, p:p + 1])
                        if rp == reps * PAIRS - 1:
                            nc.sync.dma_start(
                                loss_out[:],
                                _CACHED.pop("probe_acc")[0:1, 0:1])
                        continue
                    g_ps = gpool.tile([128, 128], F32, tag="gram")
                    for k in range(n_chunks):
                        # fp8 PE transpose requires output element step of 2
                        if dt1 is FP8:
                            pt_ps = ptpool.tile([128, 256], dt1, tag="pt")
                            pt_ap = pt_ps[:, 0:256:2]
                        else:
                            pt_ps = ptpool.tile([128, 128], dt1, tag="pt")
                            pt_ap = pt_ps[:]
                        nc.tensor.transpose(
                            pt_ap, xt_pair[:, k * CHUNK:(k + 1) * CHUNK],
                            ident_p1[:])
                        xt_sb = xtpool.tile([128, 128], dt1, tag="xt")
                        if k % 2 == 0:
                            nc.vector.tensor_copy(xt_sb[:], pt_ap)
                        else:
                            nc.scalar.copy(xt_sb[:], pt_ap)
                        nc.tensor.matmul(g_ps[:], xt_sb[:], xt_sb[:],
                                         start=(k == 0), stop=(k == n_chunks - 1))
                    # scale and ship the two diagonal 64x64 blocks (bf16)
                    gsb = gext.tile([128, 128], BF16, tag="gsb")
                    nc.scalar.mul(gsb[:], g_ps[:], gram_scale)
                    agv = ag_in.rearrange("s (c d) -> s c d", c=C)
                    nc.sync.dma_start(agv[2 * p], gsb[0:64, 0:64])
                    nc.sync.dma_start(agv[2 * p + 1], gsb[64:128, 64:128])

            # ---------------- AllGather the Grams ------------------
            if variant in ("gramsag", "full"):
                for agr in range(agreps):
                    ago = ag_out if agr == 0 else dramp.tile(
                        [B, CC], BF16, addr_space="Shared", name=f"ag_out{agr}")
                    nc.gpsimd.collective_compute(
                        "AllGather",
                        mybir.AluOpType.bypass,
                        replica_groups=[list(range(N_CORES))],
                        ins=[ag_in.opt()],
                        outs=[ago.opt()],
                    )

            if variant != "full":
                if variant in ("grams", "gramsag"):
                    with tc.tile_pool(name="dummy", bufs=1) as dummy:
                        lossd = dummy.tile([1, 1], F32)
                        nc.vector.memset(lossd[:], 0.0)
                        nc.sync.dma_start(loss_out[:], lossd[:])

            # ---------------- Phase 2: distances + reduction -------
            if variant == "full":
              with tc.tile_pool(name="p2", bufs=1) as p2, \
                 tc.tile_pool(name="at", bufs=4) as atpool, \
                 tc.tile_pool(name="atp", bufs=3, space="PSUM") as atps, \
                 tc.tile_pool(name="dotp", bufs=1, space="PSUM") as dotp, \
                 tc.tile_pool(name="smallp", bufs=1, space="PSUM") as smallp:
               for _rr in range(p2reps):
                a_sb = p2.tile([B, CC], BF16)
                nc.sync.dma_start(a_sb[:], ag_out[:])

                # dot[i,j] = <G_i, G_j> over the 4096 flattened entries
                dot_ps = dotp.tile([B, B], F32)
                for k in range(DOT_CHUNKS):
                    at_ps = atps.tile([128, B], BF16, tag="at_ps")
                    nc.tensor.transpose(
                        at_ps[:], a_sb[:, k * CHUNK:(k + 1) * CHUNK],
                        ident_bf[0:64, 0:64])
                    at_sb = atpool.tile([128, B], BF16, tag="at_sb")
                    nc.vector.tensor_copy(at_sb[:], at_ps[:])
                    nc.tensor.matmul(dot_ps[:], at_sb[:], at_sb[:],
                                     start=(k == 0), stop=(k == DOT_CHUNKS - 1))

                if tail_cut < 2:
                    cut = p2.tile([1, 1], F32, name=f"cut{_rr}")
                    nc.vector.tensor_copy(cut[:], dot_ps[0:1, 0:1])
                    nc.sync.dma_start(loss_out[:], cut[:])
                    continue
                # sq_i = diag(dot); sq_j broadcast via ones-matmul column sums
                diag_m = p2.tile([B, B], F32)
                nc.vector.tensor_tensor(diag_m[:], dot_ps[:], ident_f32[:],
                                        op=mybir.AluOpType.mult)
                sq = p2.tile([B, 1], F32)
                nc.vector.reduce_sum(sq[:], diag_m[:],
                                     axis=mybir.AxisListType.X)
                sqbc_ps = smallp.tile([B, B], F32, tag="sps")
                nc.tensor.matmul(sqbc_ps[:], ones_all[:], diag_m[:],
                                 start=True, stop=True)   # [i,j] -> sq_j
                # draw = sq_i + sq_j - 2 dot   (raw; D = draw / C^2)
                t1 = p2.tile([B, B], F32)
                nc.vector.tensor_scalar_add(t1[:], sqbc_ps[:], sq[:])
                draw = p2.tile([B, B], F32)
                nc.vector.scalar_tensor_tensor(
                    draw[:], dot_ps[:], -2.0, t1[:],
                    op0=mybir.AluOpType.mult, op1=mybir.AluOpType.add)

                if tail_cut < 3:
                    cut = p2.tile([1, 1], F32, name=f"cut{_rr}")
                    nc.vector.tensor_copy(cut[:], draw[0:1, 0:1])
                    nc.sync.dma_start(loss_out[:], cut[:])
                    continue
                # E = exp(alpha - D) restricted to negatives; Sn = rowsum
                emat = p2.tile([B, B], F32)
                nc.scalar.activation(emat[:], draw[:],
                                     mybir.ActivationFunctionType.Exp,
                                     bias=alpha_col[:], scale=-1.0 / (C * C))
                en = p2.tile([B, B], F32)
                nc.vector.tensor_tensor(en[:], emat[:], neg[:],
                                        op=mybir.AluOpType.mult)
                sn = p2.tile([B, 1], F32)
                nc.vector.reduce_sum(sn[:], en[:], axis=mybir.AxisListType.X)
                # en is symmetric, so its column sums are Sn_j as well
                snbc_ps = smallp.tile([B, B], F32, tag="sps")
                nc.tensor.matmul(snbc_ps[:], ones_all[:], en[:],
                                 start=True, stop=True)
                smat = p2.tile([B, B], F32)
                nc.vector.tensor_scalar_add(smat[:], snbc_ps[:], sn[:])

                if tail_cut < 4:
                    cut = p2.tile([1, 1], F32, name=f"cut{_rr}")
                    nc.vector.tensor_copy(cut[:], smat[0:1, 0:1])
                    nc.sync.dma_start(loss_out[:], cut[:])
                    continue
                # J = log(S) + D ; contribution = pos * relu(J)^2
                jmat = p2.tile([B, B], F32)
                nc.scalar.activation(jmat[:], smat[:],
                                     mybir.ActivationFunctionType.Ln,
                                     bias=zero_col[:], scale=1.0)
                nc.vector.scalar_tensor_tensor(
                    jmat[:], draw[:], 1.0 / (C * C), jmat[:],
                    op0=mybir.AluOpType.mult, op1=mybir.AluOpType.add)
                rmat = p2.tile([B, B], F32)
                nc.scalar.activation(rmat[:], jmat[:],
                                     mybir.ActivationFunctionType.Relu,
                                     bias=zero_col[:], scale=1.0)
                r2 = p2.tile([B, B], F32)
                nc.vector.tensor_tensor(r2[:], rmat[:], jmat[:],
                                        op=mybir.AluOpType.mult)
                scr = p2.tile([B, B], F32)
                nc.vector.tensor_tensor(scr[:], r2[:], pos[:],
                                        op=mybir.AluOpType.mult)
                nc.vector.reduce_sum(stats[:, 0:1], scr[:],
                                     axis=mybir.AxisListType.X)

                if tail_cut < 5:
                    cut = p2.tile([1, 1], F32, name=f"cut{_rr}")
                    nc.vector.tensor_copy(cut[:], scr[0:1, 0:1])
                    nc.sync.dma_start(loss_out[:], cut[:])
                    continue
                # loss = stats0 / (2 * stats1), reduced over partitions by PE
                tot_ps = smallp.tile([1, 2], F32, tag="sps1")
                nc.tensor.matmul(tot_ps[:], alpha_col[:], stats[:],
                                 start=True, stop=True)  # alpha_col == ones
                fin = p2.tile([1, 2], F32)
                nc.vector.tensor_copy(fin[:], tot_ps[:])
                invp = p2.tile([1, 1], F32)
                nc.vector.reciprocal(invp[:], fin[:, 1:2])
                lossv = p2.tile([1, 1], F32)
                nc.vector.tensor_scalar(lossv[:], fin[:, 0:1],
                                        invp[0:1, 0:1], 0.5,
                                        op0=mybir.AluOpType.mult,
                                        op1=mybir.AluOpType.mult)
                nc.sync.dma_start(loss_out[:], lossv[:])

    nc.compile()
    return nc


def _get_program():
    if "nc" not in _CACHED:
        _CACHED["nc"] = build_program()
    return _CACHED["nc"]


def run(inputs, trace=False):
    """Run on hardware; returns (loss_scalar, BassKernelResults)."""
    inp = np.ascontiguousarray(np.asarray(inputs["input"], dtype=np.float32))
    target = np.asarray(inputs["target"])
    x_full = inp.reshape(B, C, W)
    tgt_f32 = np.ascontiguousarray(
        target.astype(np.float32).reshape(1, B))

    in_maps = []
    for m in range(N_CORES):
        shard = np.ascontiguousarray(
            x_full[m * SAMP_PER_CORE:(m + 1) * SAMP_PER_CORE])
        in_maps.append({"x": shard, "tgt": tgt_f32})

    nc = _get_program()
    res = run_bass_kernel_spmd(nc, in_maps, list(range(N_CORES)), trace=trace)
    loss = np.float32(res.results[0]["loss"][0, 0])
    return loss, res


def kernel(**inputs) -> np.ndarray:
    loss, _ = run(inputs)
    return np.array(loss, dtype=np.float32)

